# revision 13
# baseline (speedup 1.0000x reference)
"""Trainium2 Bass kernel for nn_CrossAttention (4-layer MLP -> cross-attention).

Sharding: data-parallel across batch B=8, one batch element per NeuronCore.

All matmuls run in fp8(e4m3) DoubleRow (2 contraction rows per PE pass -> 2x
rate, 157 TF/s). Three algebraic folds cut the per-core matmul work from the
naive 43 GFLOP to ~31 GFLOP-equivalent:

1. Scores fold: s = q@k^T with q = h@Wq+bq, k = y@Wk+bk expands to
   h@(Wq@Wk^T)@y^T + u[q] + w[kv] + const. The per-q terms drop out of
   softmax (shift invariance along kv), so with A = Wq@Wk^T (host-folded
   weights) and w = y@(Wk@bq) (exact, host, O(S*D)) the k-projection
   disappears: z = h@A, s_eff^T = z@y^T + w[kv], w folded into the exp bias
   (per-partition in the transposed layout).

2. Value fold: out = softmax@(y@Wv+bv) needs E@(y@Wv). yv = y@Wv is
   computed once on device (fp8, requantized to fp8), then the shift trick
       E@yv = (E-1)@yv + ones (x) colsum(y@Wv)
   keeps fp8 viable: (E-1) ~ +-0.1 (scores are small) so it quantizes to
   fp8 with ~4e-3 abs error, while E itself (~1.0) would not.
   c = colsum(y)@Wv is computed on host in fp64 (O(S*D)); using the EXACT
   c (not colsum of quantized yv) cancels the correlated fp8-quantization
   bias of yv to first order (the residual is (E-1)-weighted).

3. rowsum(E) = 2048 + sum(E-1) via an all-ones [128,2,128] fp8 stationary
   matmul whose output is the rowsum replicated across all 128 partitions
   (no partition broadcast needed).

   out^T[d,q] = ((E-1)@yv)^T[d,q] + c[d]) * rinv[q] + bv[d]; output is
   stored transposed [D,S] and untransposed on host.

Layout: the MLP runs feature-major (h^T = W^T @ h^T, no transposes); scores
come out transposed (kv on partitions) and feed (E-1)@yv directly as the
moving operand against token-pair-packed yv; the result is the final out^T.
No PE transposes anywhere.

fp8 operands are pair-packed for DoubleRow: logical contraction index
k = (2t+r)*128+p lives in tile t, partition p, middle index r, i.e. SBUF
tiles [128, 2, N] (packed on host to [K/2, 2*N] so each tile is one DMA).

Engine split: scalar = relu/exp psum drains; vector = yv requant,
(E-1)->fp8, rowsum fixup, reciprocal, (x+c)*rinv; gpsimd = +bv.
All accumulation fp32 in PSUM. Fully SBUF-resident.
"""

import sys

if "/opt/trn_rl_repo" not in sys.path:
    sys.path.insert(0, "/opt/trn_rl_repo")

import numpy as np
import ml_dtypes

P = 128
D = 1024
DB = 512
S = 2048
KD = D // P       # 8 feature tiles of 128
KB = DB // P      # 4
PD = KD // 2      # 4 fp8 pair-tiles for a 1024 contraction
PB = KB // 2      # 2 for 512
NT = S // P       # 16 token tiles
NKV2 = NT // 2    # 8 token pair-tiles for the 2048 kv contraction
NB = 512          # moving-operand free-dim block
NBLK = S // NB    # 4 token blocks
HALF = S // 2     # q processed in 2 halves during attention
QB = HALF // NB   # 2 q blocks per half
NCORES = 8
SCALE = float(1.0 / np.sqrt(D))

BF16 = ml_dtypes.bfloat16
FP8 = ml_dtypes.float8_e4m3

_NC = None


def build_nc():
    """Build + compile the per-core Bass program (cached)."""
    global _NC
    if _NC is not None:
        return _NC

    from contextlib import ExitStack
    import concourse.bass as bass
    import concourse.tile as tile
    from concourse import bacc, mybir

    BF = mybir.dt.bfloat16
    F8 = mybir.dt.float8e4
    F32 = mybir.dt.float32
    AF = mybir.ActivationFunctionType
    DR = mybir.MatmulPerfMode.DoubleRow
    ADD = mybir.AluOpType.add
    MULT = mybir.AluOpType.mult

    nc = bacc.Bacc("TRN2", target_bir_lowering=False, debug=False,
                   num_devices=NCORES)

    def din(name, shape, dt):
        return nc.dram_tensor(name, shape, dt, kind="ExternalInput").ap()

    # fp8 operands arrive pair-packed: [K/2, 2*N]
    x8d = din("x8", [D // 2, 2 * S], F8)
    y8d = din("y8", [D // 2, 2 * S], F8)      # feature-major
    W1d = din("W1", [D // 2, 2 * D], F8)
    W2d = din("W2", [D // 2, 2 * DB], F8)
    W3d = din("W3", [DB // 2, 2 * D], F8)
    W4d = din("W4", [D // 2, 2 * D], F8)
    A8d = din("A8", [D // 2, 2 * D], F8)      # Wq @ Wk^T, host-folded
    Wvd = din("Wv8", [D // 2, 2 * D], F8)
    b1 = din("b1", [P, KD], F32)
    b2 = din("b2", [P, KB], F32)
    b3 = din("b3", [P, KD], F32)
    b4 = din("b4", [P, KD], F32)
    wbd = din("wb", [P, NT], F32)    # SCALE * (y @ (Wk@bq)), kv-major cols
    cbd = din("cb", [P, KD], F32)    # (colsum(y)@Wv)[d], feature-major cols
    bvd = din("bvb", [P, KD], F32)   # bv[d], feature-major cols
    outT = nc.dram_tensor("outT", [D, S], F32, kind="ExternalOutput").ap()

    with tile.TileContext(nc) as tc, ExitStack() as ctx:
        small = ctx.enter_context(tc.tile_pool(name="small", bufs=1))
        rpool = ctx.enter_context(tc.tile_pool(name="rpool", bufs=4))
        outp = ctx.enter_context(tc.tile_pool(name="outp", bufs=4))

        def load_bias(src, cols, tag):
            t = small.tile([P, cols], F32, tag=tag, name=tag)
            nc.gpsimd.dma_start(out=t, in_=src)
            return t

        b1_sb = load_bias(b1, KD, "b1")
        b2_sb = load_bias(b2, KB, "b2")
        b3_sb = load_bias(b3, KD, "b3")
        b4_sb = load_bias(b4, KD, "b4")
        wb_sb = load_bias(wbd, NT, "wb")
        cb_sb = load_bias(cbd, KD, "cb")
        bv_sb = load_bias(bvd, KD, "bv")

        ones8 = small.tile([P, 2, P], F8, tag="ones", name="ones")
        nc.vector.memset(ones8, 1.0)

        def alloc_pairs(pool, pairs, n, tag, dt=F8):
            """fp8 pair-packed tiles [P, 2, n]."""
            return [pool.tile([P, 2, n], dt, tag=f"{tag}{t}", name=f"{tag}{t}")
                    for t in range(pairs)]

        def load_pairs(tiles, src, n):
            for t, tl in enumerate(tiles):
                nc.sync.dma_start(
                    out=tl,
                    in_=src[t * P:(t + 1) * P, :].rearrange(
                        "p (r s) -> p r s", r=2))

        def fm_layer8(psum, src8, w8, pairs, mtiles, bias_sb, func, dst8):
            """fp8 DoubleRow feature-major layer into pair-packed fp8 dst."""
            for m in range(mtiles):
                pss = [psum.tile([P, NB], F32, tag="mm", name="mm")
                       for _ in range(NBLK)]
                for t in range(pairs):
                    lhs = w8[t][:, :, m * P:(m + 1) * P]
                    for tb in range(NBLK):
                        nc.tensor.matmul(pss[tb], lhs,
                                         src8[t][:, :, tb * NB:(tb + 1) * NB],
                                         start=(t == 0), stop=(t == pairs - 1),
                                         perf_mode=DR)
                for tb in range(NBLK):
                    dst = dst8[m // 2][:, m % 2, tb * NB:(tb + 1) * NB]
                    nc.scalar.activation(
                        dst, pss[tb], func,
                        bias=0.0 if bias_sb is None else bias_sb[:, m:m + 1],
                        scale=1.0)

        # ------ persistent attention operands + y prefetch ------
        with tc.tile_pool(name="pz", bufs=1) as pz, \
             tc.tile_pool(name="py", bufs=1) as py, \
             tc.tile_pool(name="pyv", bufs=1) as pyv, \
             tc.tile_pool(name="pwv", bufs=1) as pwv:
            z8 = alloc_pairs(pz, PD, S, "z8")
            y8 = alloc_pairs(py, PD, S, "y8")
            yv8t = alloc_pairs(pyv, NKV2, D, "yv8t")
            wv8 = alloc_pairs(pwv, PD, D, "wv8")

            # ---------------- Stage A: x-MLP -> z8 (in SBUF) ----------------
            with tc.tile_pool(name="wx", bufs=1) as wx, \
                 tc.tile_pool(name="px", bufs=1) as px, \
                 tc.tile_pool(name="phA", bufs=1) as phA, \
                 tc.tile_pool(name="phB", bufs=1) as phB, \
                 tc.tile_pool(name="psA", bufs=8, space="PSUM") as psA:
                x8 = alloc_pairs(px, PD, S, "x8")
                w18 = alloc_pairs(wx, PD, D, "w18")
                # L1's first psum group (m=0, tb=0) needs only the first
                # m/tb chunk of every pair tile -- land those chunks first
                # so the PE starts ~3us earlier, then stream the rest.
                x8r = x8d.rearrange("k (r s) -> k r s", r=2)
                w1r = W1d.rearrange("k (r s) -> k r s", r=2)
                for t in range(PD):
                    sl = slice(t * P, (t + 1) * P)
                    nc.sync.dma_start(out=w18[t][:, :, 0:P],
                                      in_=w1r[sl, :, 0:P])
                    nc.sync.dma_start(out=x8[t][:, :, 0:NB],
                                      in_=x8r[sl, :, 0:NB])
                for tb in range(1, NBLK):
                    for t in range(PD):
                        sl = slice(t * P, (t + 1) * P)
                        nc.sync.dma_start(
                            out=x8[t][:, :, tb * NB:(tb + 1) * NB],
                            in_=x8r[sl, :, tb * NB:(tb + 1) * NB])
                for t in range(PD):
                    sl = slice(t * P, (t + 1) * P)
                    nc.sync.dma_start(out=w18[t][:, :, P:D],
                                      in_=w1r[sl, :, P:D])
                w28 = alloc_pairs(wx, PD, DB, "w28")
                load_pairs(w28, W2d, DB)
                w38 = alloc_pairs(wx, PB, D, "w38")
                load_pairs(w38, W3d, D)
                w48 = alloc_pairs(wx, PD, D, "w48")
                load_pairs(w48, W4d, D)
                a8 = alloc_pairs(wx, PD, D, "a8")
                load_pairs(a8, A8d, D)
                # y-side prefetch (queued behind stage A's needs)
                load_pairs(y8, y8d, S)
                load_pairs(wv8, Wvd, D)

                h18 = alloc_pairs(phA, PD, S, "ha")
                h28 = alloc_pairs(phB, PB, S, "hb")
                h38 = alloc_pairs(phA, PD, S, "ha")   # reuse phA slots
                h48 = alloc_pairs(phB, PD, S, "hb")   # grow phB to 4 pair slots
                fm_layer8(psA, x8, w18, PD, KD, b1_sb, AF.Relu, h18)
                fm_layer8(psA, h18, w28, PD, KB, b2_sb, AF.Relu, h28)
                fm_layer8(psA, h28, w38, PB, KD, b3_sb, AF.Relu, h38)
                fm_layer8(psA, h38, w48, PD, KD, b4_sb, AF.Relu, h48)
                fm_layer8(psA, h48, a8, PD, KD, None, AF.Identity, z8)

            # ------------ Stage B: yv = y@Wv (fp8, requant to kv-pairs) -----
            with tc.tile_pool(name="psBv", bufs=4, space="PSUM") as psBv:
                for tkv in range(NT):
                    for db in range(2):
                        pv = psBv.tile([P, NB], F32, tag="vv", name="vv")
                        for t in range(PD):
                            nc.tensor.matmul(
                                pv, y8[t][:, :, tkv * P:(tkv + 1) * P],
                                wv8[t][:, :, db * NB:(db + 1) * NB],
                                start=(t == 0), stop=(t == PD - 1),
                                perf_mode=DR)
                        nc.vector.tensor_copy(
                            out=yv8t[tkv // 2][:, tkv % 2,
                                               db * NB:(db + 1) * NB],
                            in_=pv)

            # ---------------- Stage C: attention ----------------
            with tc.tile_pool(name="pE", bufs=2) as pE, \
                 tc.tile_pool(name="pT", bufs=4) as pT, \
                 tc.tile_pool(name="psCs", bufs=4, space="PSUM") as psCs, \
                 tc.tile_pool(name="psEY", bufs=3, space="PSUM") as psEY, \
                 tc.tile_pool(name="psRS", bufs=1, space="PSUM") as psRS:
                for half in range(2):
                    qoff = half * HALF
                    et1 = alloc_pairs(pE, NKV2, HALF, "e")
                    # scores^T -> E-1 in fp8, kv pair-packed, per q block
                    for qb in range(QB):
                        for tk in range(NT):
                            ps = psCs.tile([P, NB], F32, tag="sc", name="sc")
                            for t in range(PD):
                                nc.tensor.matmul(
                                    ps, y8[t][:, :, tk * P:(tk + 1) * P],
                                    z8[t][:, :, qoff + qb * NB:
                                          qoff + (qb + 1) * NB],
                                    start=(t == 0), stop=(t == PD - 1),
                                    perf_mode=DR)
                            etmp = pT.tile([P, NB], BF, tag="et", name="et")
                            nc.scalar.activation(etmp, ps, AF.Exp,
                                                 bias=wb_sb[:, tk:tk + 1],
                                                 scale=SCALE)
                            nc.vector.tensor_scalar_add(
                                et1[tk // 2][:, tk % 2,
                                             qb * NB:(qb + 1) * NB],
                                etmp, -1.0)
                    for qb in range(QB):
                        # rowsum(E) = 2048 + sum(E-1), replicated on all
                        # partitions via the all-ones stationary
                        prs = psRS.tile([P, NB], F32, tag="rs", name="rs")
                        for t2 in range(NKV2):
                            nc.tensor.matmul(
                                prs, ones8,
                                et1[t2][:, :, qb * NB:(qb + 1) * NB],
                                start=(t2 == 0), stop=(t2 == NKV2 - 1),
                                perf_mode=DR)
                        rsf = rpool.tile([P, NB], F32, tag="rf", name="rf")
                        nc.vector.tensor_scalar_add(rsf, prs, float(S))
                        rrep = rpool.tile([P, NB], F32, tag=f"rp{qb}",
                                          name=f"rp{qb}")
                        nc.vector.reciprocal(rrep, rsf)
                        # out^T = ((E-1)@yv + c) * rinv + bv
                        for dt_ in range(KD):
                            po = psEY.tile([P, NB], F32, tag="oo", name="oo")
                            for t2 in range(NKV2):
                                nc.tensor.matmul(
                                    po, yv8t[t2][:, :, dt_ * P:(dt_ + 1) * P],
                                    et1[t2][:, :, qb * NB:(qb + 1) * NB],
                                    start=(t2 == 0), stop=(t2 == NKV2 - 1),
                                    perf_mode=DR)
                            tmp = pT.tile([P, NB], F32, tag="tm", name="tm")
                            nc.vector.scalar_tensor_tensor(
                                tmp, po, cb_sb[:, dt_:dt_ + 1], rrep,
                                op0=ADD, op1=MULT)
                            ot = outp.tile([P, NB], F32, tag="ot", name="ot")
                            nc.scalar.activation(ot, tmp, AF.Identity,
                                                 bias=bv_sb[:, dt_:dt_ + 1],
                                                 scale=1.0)
                            nc.sync.dma_start(
                                out=outT[dt_ * P:(dt_ + 1) * P,
                                         qoff + qb * NB:qoff + (qb + 1) * NB],
                                in_=ot)

    nc.compile()
    _NC = nc
    return nc


def _pack8(w):
    """[K, N] -> DoubleRow pair-packed fp8 [K/2, 2N]:
    out[t*128+p, r*N+m] = w[(2t+r)*128+p, m]."""
    K, N = w.shape
    return np.ascontiguousarray(
        w.astype(FP8).reshape(K // 256, 2, 128, N)
        .transpose(0, 2, 1, 3).reshape(K // 2, 2 * N))


def make_in_maps(inputs):
    """Host-side prep: per-core batch shard, fp8 casts + pair packing,
    feature-major transposes of x/y, weight folding A = Wq@Wk^T, and the
    exact O(S*D) correction vectors w = SCALE*(y@(Wk@bq)), c = colsum(y)@Wv.
    """
    x = np.asarray(inputs["x"])
    y = np.asarray(inputs["y"])
    Wq = np.asarray(inputs["Wq"]).astype(np.float64)
    Wk = np.asarray(inputs["Wk"]).astype(np.float64)
    Wv = np.asarray(inputs["Wv"]).astype(np.float64)
    bq = np.asarray(inputs["bq"]).astype(np.float64)
    A = (Wq @ Wk.T).astype(np.float32)
    wk_bq = Wk @ bq
    shared = {}
    for k in ("W1", "W2", "W3", "W4"):
        shared[k] = _pack8(np.asarray(inputs[k]).astype(np.float32))
    shared["A8"] = _pack8(A)
    shared["Wv8"] = _pack8(np.asarray(inputs["Wv"]).astype(np.float32))
    for k, nt in (("b1", KD), ("b2", KB), ("b3", KD), ("b4", KD)):
        shared[k] = np.ascontiguousarray(
            np.asarray(inputs[k]).astype(np.float32).reshape(nt, P).T)
    shared["bvb"] = np.ascontiguousarray(
        np.asarray(inputs["bv"]).astype(np.float32).reshape(KD, P).T)
    in_maps = []
    for b in range(x.shape[0]):
        m = dict(shared)
        m["x8"] = _pack8(np.ascontiguousarray(x[b].T))
        m["y8"] = _pack8(np.ascontiguousarray(y[b].T))
        yb = y[b].astype(np.float64)
        m["wb"] = np.ascontiguousarray(
            (SCALE * (yb @ wk_bq)).astype(np.float32).reshape(NT, P).T)
        m["cb"] = np.ascontiguousarray(
            (yb.sum(0) @ Wv).astype(np.float32).reshape(KD, P).T)
        in_maps.append(m)
    return in_maps


def kernel(**inputs):
    from concourse.bass_utils import run_bass_kernel_spmd

    nc = build_nc()
    in_maps = make_in_maps(inputs)
    res = run_bass_kernel_spmd(nc, in_maps, list(range(len(in_maps))))
    return np.stack([np.asarray(r["outT"], dtype=np.float32).T
                     for r in res.results])


# revision 17
# speedup vs baseline: 1.0119x; 1.0119x over previous
"""Trainium2 Bass kernel for nn_CrossAttention (4-layer MLP -> cross-attention).

Sharding: data-parallel across batch B=8, one batch element per NeuronCore.

All matmuls run in fp8(e4m3) DoubleRow (2 contraction rows per PE pass -> 2x
rate, 157 TF/s). Three algebraic folds cut the per-core matmul work from the
naive 43 GFLOP to ~31 GFLOP-equivalent:

1. Scores fold: s = q@k^T with q = h@Wq+bq, k = y@Wk+bk expands to
   h@(Wq@Wk^T)@y^T + u[q] + w[kv] + const. The per-q terms drop out of
   softmax (shift invariance along kv), so with A = Wq@Wk^T (host-folded
   weights) and w = y@(Wk@bq) (exact, host, O(S*D)) the k-projection
   disappears: z = h@A, s_eff^T = z@y^T + w[kv], w folded into the exp bias
   (per-partition in the transposed layout).

2. Value fold: out = softmax@(y@Wv+bv) needs E@(y@Wv). yv = y@Wv is
   computed once on device (fp8, requantized to fp8), then the shift trick
       E@yv = (E-1)@yv + ones (x) colsum(y@Wv)
   keeps fp8 viable: (E-1) ~ +-0.1 (scores are small) so it quantizes to
   fp8 with ~4e-3 abs error, while E itself (~1.0) would not.
   c = colsum(y)@Wv is computed on host in fp64 (O(S*D)); using the EXACT
   c (not colsum of quantized yv) cancels the correlated fp8-quantization
   bias of yv to first order (the residual is (E-1)-weighted).

3. rowsum(E) = 2048 + sum(E-1) via an all-ones [128,2,128] fp8 stationary
   matmul whose output is the rowsum replicated across all 128 partitions
   (no partition broadcast needed).

   out^T[d,q] = ((E-1)@yv)^T[d,q] + c[d]) * rinv[q] + bv[d]; output is
   stored transposed [D,S] and untransposed on host.

Layout: the MLP runs feature-major (h^T = W^T @ h^T, no transposes); scores
come out transposed (kv on partitions) and feed (E-1)@yv directly as the
moving operand against token-pair-packed yv; the result is the final out^T.
No PE transposes anywhere.

fp8 operands are pair-packed for DoubleRow: logical contraction index
k = (2t+r)*128+p lives in tile t, partition p, middle index r, i.e. SBUF
tiles [128, 2, N] (packed on host to [K/2, 2*N] so each tile is one DMA).

Engine split: scalar = relu/exp psum drains; vector = yv requant,
(E-1)->fp8, rowsum fixup, reciprocal, (x+c)*rinv; gpsimd = +bv.
All accumulation fp32 in PSUM. Fully SBUF-resident.
"""

import sys

if "/opt/trn_rl_repo" not in sys.path:
    sys.path.insert(0, "/opt/trn_rl_repo")

import numpy as np
import ml_dtypes

P = 128
D = 1024
DB = 512
S = 2048
KD = D // P       # 8 feature tiles of 128
KB = DB // P      # 4
PD = KD // 2      # 4 fp8 pair-tiles for a 1024 contraction
PB = KB // 2      # 2 for 512
NT = S // P       # 16 token tiles
NKV2 = NT // 2    # 8 token pair-tiles for the 2048 kv contraction
NB = 512          # moving-operand free-dim block
NBLK = S // NB    # 4 token blocks
HALF = S // 2     # q processed in 2 halves during attention
QB = HALF // NB   # 2 q blocks per half
NCORES = 8
SCALE = float(1.0 / np.sqrt(D))

BF16 = ml_dtypes.bfloat16
FP8 = ml_dtypes.float8_e4m3

_NC = None


def build_nc():
    """Build + compile the per-core Bass program (cached)."""
    global _NC
    if _NC is not None:
        return _NC

    from contextlib import ExitStack
    import concourse.bass as bass
    import concourse.tile as tile
    from concourse import bacc, mybir

    BF = mybir.dt.bfloat16
    F8 = mybir.dt.float8e4
    F32 = mybir.dt.float32
    AF = mybir.ActivationFunctionType
    DR = mybir.MatmulPerfMode.DoubleRow
    ADD = mybir.AluOpType.add
    MULT = mybir.AluOpType.mult

    nc = bacc.Bacc("TRN2", target_bir_lowering=False, debug=False,
                   num_devices=NCORES)

    def din(name, shape, dt):
        return nc.dram_tensor(name, shape, dt, kind="ExternalInput").ap()

    # fp8 operands arrive pair-packed: [K/2, 2*N]
    x8d = din("x8", [D // 2, 2 * S], F8)
    y8d = din("y8", [D // 2, 2 * S], F8)      # feature-major
    W1d = din("W1", [D // 2, 2 * D], F8)
    W2d = din("W2", [D // 2, 2 * DB], F8)
    W3d = din("W3", [DB // 2, 2 * D], F8)
    W4d = din("W4", [D // 2, 2 * D], F8)
    A8d = din("A8", [D // 2, 2 * D], F8)      # Wq @ Wk^T, host-folded
    Wvd = din("Wv8", [D // 2, 2 * D], F8)
    b1 = din("b1", [P, KD], F32)
    b2 = din("b2", [P, KB], F32)
    b3 = din("b3", [P, KD], F32)
    b4 = din("b4", [P, KD], F32)
    wbd = din("wb", [P, NT], F32)    # SCALE * (y @ (Wk@bq)), kv-major cols
    cbd = din("cb", [P, KD], F32)    # (colsum(y)@Wv)[d], feature-major cols
    bvd = din("bvb", [P, KD], F32)   # bv[d], feature-major cols
    outT = nc.dram_tensor("outT", [D, S], F32, kind="ExternalOutput").ap()

    with tile.TileContext(nc) as tc, ExitStack() as ctx:
        small = ctx.enter_context(tc.tile_pool(name="small", bufs=1))
        rpool = ctx.enter_context(tc.tile_pool(name="rpool", bufs=4))
        outp = ctx.enter_context(tc.tile_pool(name="outp", bufs=4))

        def load_bias(src, cols, tag):
            t = small.tile([P, cols], F32, tag=tag, name=tag)
            nc.gpsimd.dma_start(out=t, in_=src)
            return t

        b1_sb = load_bias(b1, KD, "b1")
        b2_sb = load_bias(b2, KB, "b2")
        b3_sb = load_bias(b3, KD, "b3")
        b4_sb = load_bias(b4, KD, "b4")
        wb_sb = load_bias(wbd, NT, "wb")
        cb_sb = load_bias(cbd, KD, "cb")
        bv_sb = load_bias(bvd, KD, "bv")

        ones8 = small.tile([P, 2, P], F8, tag="ones", name="ones")
        nc.vector.memset(ones8, 1.0)

        def alloc_pairs(pool, pairs, n, tag, dt=F8):
            """fp8 pair-packed tiles [P, 2, n]."""
            return [pool.tile([P, 2, n], dt, tag=f"{tag}{t}", name=f"{tag}{t}")
                    for t in range(pairs)]

        def load_pairs(tiles, src, n):
            for t, tl in enumerate(tiles):
                nc.sync.dma_start(
                    out=tl,
                    in_=src[t * P:(t + 1) * P, :].rearrange(
                        "p (r s) -> p r s", r=2))

        def fm_layer8(psum, src8, w8, pairs, mtiles, bias_sb, func, dst8,
                      tb_outer=False):
            """fp8 DoubleRow feature-major layer into pair-packed fp8 dst.

            tb_outer runs token blocks in the outer loop so each block only
            needs 1/NBLK of src8 -- used for L1 whose src is still in
            flight from DRAM when compute starts."""
            outer, inner = ((NBLK, mtiles) if tb_outer else (mtiles, NBLK))
            for o in range(outer):
                pss = [psum.tile([P, NB], F32, tag="mm", name="mm")
                       for _ in range(inner)]
                for t in range(pairs):
                    for i in range(inner):
                        m, tb = (i, o) if tb_outer else (o, i)
                        nc.tensor.matmul(
                            pss[i], w8[t][:, :, m * P:(m + 1) * P],
                            src8[t][:, :, tb * NB:(tb + 1) * NB],
                            start=(t == 0), stop=(t == pairs - 1),
                            perf_mode=DR)
                for i in range(inner):
                    m, tb = (i, o) if tb_outer else (o, i)
                    dst = dst8[m // 2][:, m % 2, tb * NB:(tb + 1) * NB]
                    nc.scalar.activation(
                        dst, pss[i], func,
                        bias=0.0 if bias_sb is None else bias_sb[:, m:m + 1],
                        scale=1.0)

        # ------ persistent attention operands + y prefetch ------
        with tc.tile_pool(name="pz", bufs=1) as pz, \
             tc.tile_pool(name="py", bufs=1) as py, \
             tc.tile_pool(name="pyv", bufs=1) as pyv, \
             tc.tile_pool(name="pwv", bufs=1) as pwv:
            z8 = alloc_pairs(pz, PD, S, "z8")
            y8 = alloc_pairs(py, PD, S, "y8")
            yv8t = alloc_pairs(pyv, NKV2, D, "yv8t")
            wv8 = alloc_pairs(pwv, PD, D, "wv8")

            # ---------------- Stage A: x-MLP -> z8 (in SBUF) ----------------
            with tc.tile_pool(name="wx", bufs=1) as wx, \
                 tc.tile_pool(name="px", bufs=1) as px, \
                 tc.tile_pool(name="phA", bufs=1) as phA, \
                 tc.tile_pool(name="phB", bufs=1) as phB, \
                 tc.tile_pool(name="psA", bufs=8, space="PSUM") as psA:
                x8 = alloc_pairs(px, PD, S, "x8")
                w18 = alloc_pairs(wx, PD, D, "w18")
                # L1 runs token-block-outer: block tb only needs x8[*][tb],
                # so land full W1 + the first x8 block first, then stream
                # the remaining x8 blocks ahead of their use.
                x8r = x8d.rearrange("k (r s) -> k r s", r=2)
                for t in range(PD):
                    sl = slice(t * P, (t + 1) * P)
                    nc.sync.dma_start(
                        out=w18[t], in_=W1d[sl, :].rearrange(
                            "p (r s) -> p r s", r=2))
                    nc.sync.dma_start(out=x8[t][:, :, 0:NB],
                                      in_=x8r[sl, :, 0:NB])
                for tb in range(1, NBLK):
                    for t in range(PD):
                        sl = slice(t * P, (t + 1) * P)
                        nc.sync.dma_start(
                            out=x8[t][:, :, tb * NB:(tb + 1) * NB],
                            in_=x8r[sl, :, tb * NB:(tb + 1) * NB])
                w28 = alloc_pairs(wx, PD, DB, "w28")
                load_pairs(w28, W2d, DB)
                w38 = alloc_pairs(wx, PB, D, "w38")
                load_pairs(w38, W3d, D)
                w48 = alloc_pairs(wx, PD, D, "w48")
                load_pairs(w48, W4d, D)
                a8 = alloc_pairs(wx, PD, D, "a8")
                load_pairs(a8, A8d, D)
                # y-side prefetch (queued behind stage A's needs)
                load_pairs(y8, y8d, S)
                load_pairs(wv8, Wvd, D)

                h18 = alloc_pairs(phA, PD, S, "ha")
                h28 = alloc_pairs(phB, PB, S, "hb")
                h38 = alloc_pairs(phA, PD, S, "ha")   # reuse phA slots
                h48 = alloc_pairs(phB, PD, S, "hb")   # grow phB to 4 pair slots
                fm_layer8(psA, x8, w18, PD, KD, b1_sb, AF.Relu, h18,
                          tb_outer=True)
                fm_layer8(psA, h18, w28, PD, KB, b2_sb, AF.Relu, h28)
                fm_layer8(psA, h28, w38, PB, KD, b3_sb, AF.Relu, h38)
                fm_layer8(psA, h38, w48, PD, KD, b4_sb, AF.Relu, h48)
                fm_layer8(psA, h48, a8, PD, KD, None, AF.Identity, z8)

            # ------------ Stage B: yv = y@Wv (fp8, requant to kv-pairs) -----
            with tc.tile_pool(name="psBv", bufs=4, space="PSUM") as psBv:
                for tkv in range(NT):
                    for db in range(2):
                        pv = psBv.tile([P, NB], F32, tag="vv", name="vv")
                        for t in range(PD):
                            nc.tensor.matmul(
                                pv, y8[t][:, :, tkv * P:(tkv + 1) * P],
                                wv8[t][:, :, db * NB:(db + 1) * NB],
                                start=(t == 0), stop=(t == PD - 1),
                                perf_mode=DR)
                        nc.vector.tensor_copy(
                            out=yv8t[tkv // 2][:, tkv % 2,
                                               db * NB:(db + 1) * NB],
                            in_=pv)

            # ---------------- Stage C: attention (single pass) ----------------
            with tc.tile_pool(name="pE", bufs=1) as pE, \
                 tc.tile_pool(name="pT", bufs=4) as pT, \
                 tc.tile_pool(name="psCs", bufs=5, space="PSUM") as psCs, \
                 tc.tile_pool(name="psEY", bufs=2, space="PSUM") as psEY, \
                 tc.tile_pool(name="psRS", bufs=1, space="PSUM") as psRS:
                et1 = alloc_pairs(pE, NKV2, S, "e")
                # scores^T -> E-1 in fp8, kv pair-packed, per q block
                for qb in range(NBLK):
                    for tk in range(NT):
                        ps = psCs.tile([P, NB], F32, tag="sc", name="sc")
                        for t in range(PD):
                            nc.tensor.matmul(
                                ps, y8[t][:, :, tk * P:(tk + 1) * P],
                                z8[t][:, :, qb * NB:(qb + 1) * NB],
                                start=(t == 0), stop=(t == PD - 1),
                                perf_mode=DR)
                        etmp = pT.tile([P, NB], BF, tag="et", name="et")
                        nc.scalar.activation(etmp, ps, AF.Exp,
                                             bias=wb_sb[:, tk:tk + 1],
                                             scale=SCALE)
                        nc.vector.tensor_scalar_add(
                            et1[tk // 2][:, tk % 2, qb * NB:(qb + 1) * NB],
                            etmp, -1.0)
                for qb in range(NBLK):
                    # rowsum(E) = 2048 + sum(E-1), replicated on all
                    # partitions via the all-ones stationary
                    prs = psRS.tile([P, NB], F32, tag="rs", name="rs")
                    for t2 in range(NKV2):
                        nc.tensor.matmul(
                            prs, ones8,
                            et1[t2][:, :, qb * NB:(qb + 1) * NB],
                            start=(t2 == 0), stop=(t2 == NKV2 - 1),
                            perf_mode=DR)
                    rsf = rpool.tile([P, NB], F32, tag="rf", name="rf")
                    nc.vector.tensor_scalar_add(rsf, prs, float(S))
                    rrep = rpool.tile([P, NB], F32, tag=f"rp{qb % 2}",
                                      name=f"rp{qb % 2}")
                    nc.vector.reciprocal(rrep, rsf)
                    # out^T = ((E-1)@yv + c) * rinv + bv
                    for dt_ in range(KD):
                        po = psEY.tile([P, NB], F32, tag="oo", name="oo")
                        for t2 in range(NKV2):
                            nc.tensor.matmul(
                                po, yv8t[t2][:, :, dt_ * P:(dt_ + 1) * P],
                                et1[t2][:, :, qb * NB:(qb + 1) * NB],
                                start=(t2 == 0), stop=(t2 == NKV2 - 1),
                                perf_mode=DR)
                        tmp = pT.tile([P, NB], F32, tag="tm", name="tm")
                        nc.vector.scalar_tensor_tensor(
                            tmp, po, cb_sb[:, dt_:dt_ + 1], rrep,
                            op0=ADD, op1=MULT)
                        ot = outp.tile([P, NB], F32, tag="ot", name="ot")
                        nc.scalar.activation(ot, tmp, AF.Identity,
                                             bias=bv_sb[:, dt_:dt_ + 1],
                                             scale=1.0)
                        nc.sync.dma_start(
                            out=outT[dt_ * P:(dt_ + 1) * P,
                                     qb * NB:(qb + 1) * NB],
                            in_=ot)

    nc.compile()
    _NC = nc
    return nc


def _pack8(w):
    """[K, N] -> DoubleRow pair-packed fp8 [K/2, 2N]:
    out[t*128+p, r*N+m] = w[(2t+r)*128+p, m]."""
    K, N = w.shape
    return np.ascontiguousarray(
        w.astype(FP8).reshape(K // 256, 2, 128, N)
        .transpose(0, 2, 1, 3).reshape(K // 2, 2 * N))


def make_in_maps(inputs):
    """Host-side prep: per-core batch shard, fp8 casts + pair packing,
    feature-major transposes of x/y, weight folding A = Wq@Wk^T, and the
    exact O(S*D) correction vectors w = SCALE*(y@(Wk@bq)), c = colsum(y)@Wv.
    """
    x = np.asarray(inputs["x"])
    y = np.asarray(inputs["y"])
    Wq = np.asarray(inputs["Wq"]).astype(np.float64)
    Wk = np.asarray(inputs["Wk"]).astype(np.float64)
    Wv = np.asarray(inputs["Wv"]).astype(np.float64)
    bq = np.asarray(inputs["bq"]).astype(np.float64)
    A = (Wq @ Wk.T).astype(np.float32)
    wk_bq = Wk @ bq
    shared = {}
    for k in ("W1", "W2", "W3", "W4"):
        shared[k] = _pack8(np.asarray(inputs[k]).astype(np.float32))
    shared["A8"] = _pack8(A)
    shared["Wv8"] = _pack8(np.asarray(inputs["Wv"]).astype(np.float32))
    for k, nt in (("b1", KD), ("b2", KB), ("b3", KD), ("b4", KD)):
        shared[k] = np.ascontiguousarray(
            np.asarray(inputs[k]).astype(np.float32).reshape(nt, P).T)
    shared["bvb"] = np.ascontiguousarray(
        np.asarray(inputs["bv"]).astype(np.float32).reshape(KD, P).T)
    in_maps = []
    for b in range(x.shape[0]):
        m = dict(shared)
        m["x8"] = _pack8(np.ascontiguousarray(x[b].T))
        m["y8"] = _pack8(np.ascontiguousarray(y[b].T))
        yb = y[b].astype(np.float64)
        m["wb"] = np.ascontiguousarray(
            (SCALE * (yb @ wk_bq)).astype(np.float32).reshape(NT, P).T)
        m["cb"] = np.ascontiguousarray(
            (yb.sum(0) @ Wv).astype(np.float32).reshape(KD, P).T)
        in_maps.append(m)
    return in_maps


def kernel(**inputs):
    from concourse.bass_utils import run_bass_kernel_spmd

    nc = build_nc()
    in_maps = make_in_maps(inputs)
    res = run_bass_kernel_spmd(nc, in_maps, list(range(len(in_maps))))
    return np.stack([np.asarray(r["outT"], dtype=np.float32).T
                     for r in res.results])


# revision 20
# speedup vs baseline: 1.0182x; 1.0062x over previous
"""Trainium2 Bass kernel for nn_CrossAttention (4-layer MLP -> cross-attention).

Sharding: data-parallel across batch B=8, one batch element per NeuronCore.

All matmuls run in fp8(e4m3) DoubleRow (2 contraction rows per PE pass -> 2x
rate, 157 TF/s). Three algebraic folds cut the per-core matmul work from the
naive 43 GFLOP to ~31 GFLOP-equivalent:

1. Scores fold: s = q@k^T with q = h@Wq+bq, k = y@Wk+bk expands to
   h@(Wq@Wk^T)@y^T + u[q] + w[kv] + const. The per-q terms drop out of
   softmax (shift invariance along kv), so with A = Wq@Wk^T (host-folded
   weights) and w = y@(Wk@bq) (exact, host, O(S*D)) the k-projection
   disappears: z = h@A, s_eff^T = z@y^T + w[kv], w folded into the exp bias
   (per-partition in the transposed layout).

2. Value fold: out = softmax@(y@Wv+bv) needs E@(y@Wv). yv = y@Wv is
   computed once on device (fp8, requantized to fp8), then the shift trick
       E@yv = (E-1)@yv + ones (x) colsum(y@Wv)
   keeps fp8 viable: (E-1) ~ +-0.1 (scores are small) so it quantizes to
   fp8 with ~4e-3 abs error, while E itself (~1.0) would not.
   c = colsum(y)@Wv is computed on host in fp64 (O(S*D)); using the EXACT
   c (not colsum of quantized yv) cancels the correlated fp8-quantization
   bias of yv to first order (the residual is (E-1)-weighted).

3. rowsum(E) = 2048 + sum(E-1) via an all-ones [128,2,128] fp8 stationary
   matmul whose output is the rowsum replicated across all 128 partitions
   (no partition broadcast needed).

   out^T[d,q] = ((E-1)@yv)^T[d,q] + c[d]) * rinv[q] + bv[d]; output is
   stored transposed [D,S] and untransposed on host.

Layout: the MLP runs feature-major (h^T = W^T @ h^T, no transposes); scores
come out transposed (kv on partitions) and feed (E-1)@yv directly as the
moving operand against token-pair-packed yv; the result is the final out^T.
No PE transposes anywhere.

fp8 operands are pair-packed for DoubleRow: logical contraction index
k = (2t+r)*128+p lives in tile t, partition p, middle index r, i.e. SBUF
tiles [128, 2, N] (packed on host to [K/2, 2*N] so each tile is one DMA).

Engine split: scalar = relu/exp psum drains; vector = yv requant,
(E-1)->fp8, rowsum fixup, reciprocal, (x+c)*rinv; gpsimd = +bv.
All accumulation fp32 in PSUM. Fully SBUF-resident.
"""

import sys

if "/opt/trn_rl_repo" not in sys.path:
    sys.path.insert(0, "/opt/trn_rl_repo")

import numpy as np
import ml_dtypes

P = 128
D = 1024
DB = 512
S = 2048
KD = D // P       # 8 feature tiles of 128
KB = DB // P      # 4
PD = KD // 2      # 4 fp8 pair-tiles for a 1024 contraction
PB = KB // 2      # 2 for 512
NT = S // P       # 16 token tiles
NKV2 = NT // 2    # 8 token pair-tiles for the 2048 kv contraction
NB = 512          # moving-operand free-dim block
NBLK = S // NB    # 4 token blocks
HALF = S // 2     # q processed in 2 halves during attention
QB = HALF // NB   # 2 q blocks per half
NCORES = 8
SCALE = float(1.0 / np.sqrt(D))

BF16 = ml_dtypes.bfloat16
FP8 = ml_dtypes.float8_e4m3

_NC = None


def build_nc():
    """Build + compile the per-core Bass program (cached)."""
    global _NC
    if _NC is not None:
        return _NC

    from contextlib import ExitStack
    import concourse.bass as bass
    import concourse.tile as tile
    from concourse import bacc, mybir

    BF = mybir.dt.bfloat16
    F8 = mybir.dt.float8e4
    F32 = mybir.dt.float32
    AF = mybir.ActivationFunctionType
    DR = mybir.MatmulPerfMode.DoubleRow
    ADD = mybir.AluOpType.add
    MULT = mybir.AluOpType.mult

    nc = bacc.Bacc("TRN2", target_bir_lowering=False, debug=False,
                   num_devices=NCORES)

    def din(name, shape, dt):
        return nc.dram_tensor(name, shape, dt, kind="ExternalInput").ap()

    # fp8 operands arrive pair-packed: [K/2, 2*N]
    x8d = din("x8", [D // 2, 2 * S], F8)
    y8d = din("y8", [D // 2, 2 * S], F8)      # feature-major
    W1d = din("W1", [D // 2, 2 * D], F8)
    W2d = din("W2", [D // 2, 2 * DB], F8)
    W3d = din("W3", [DB // 2, 2 * D], F8)
    W4d = din("W4", [D // 2, 2 * D], F8)
    A8d = din("A8", [D // 2, 2 * D], F8)      # Wq @ Wk^T, host-folded
    Wvd = din("Wv8", [D // 2, 2 * D], F8)
    b1 = din("b1", [P, KD], F32)
    b2 = din("b2", [P, KB], F32)
    b3 = din("b3", [P, KD], F32)
    b4 = din("b4", [P, KD], F32)
    wbd = din("wb", [P, NT], F32)    # SCALE * (y @ (Wk@bq)), kv-major cols
    cbd = din("cb", [P, KD], F32)    # (colsum(y)@Wv)[d], feature-major cols
    bvd = din("bvb", [P, KD], F32)   # bv[d], feature-major cols
    outT = nc.dram_tensor("outT", [D, S], F32, kind="ExternalOutput").ap()

    with tile.TileContext(nc) as tc, ExitStack() as ctx:
        small = ctx.enter_context(tc.tile_pool(name="small", bufs=1))
        rpool = ctx.enter_context(tc.tile_pool(name="rpool", bufs=4))
        outp = ctx.enter_context(tc.tile_pool(name="outp", bufs=8))

        def load_bias(src, cols, tag):
            t = small.tile([P, cols], F32, tag=tag, name=tag)
            nc.gpsimd.dma_start(out=t, in_=src)
            return t

        b1_sb = load_bias(b1, KD, "b1")
        b2_sb = load_bias(b2, KB, "b2")
        b3_sb = load_bias(b3, KD, "b3")
        b4_sb = load_bias(b4, KD, "b4")
        wb_sb = load_bias(wbd, NT, "wb")
        cb_sb = load_bias(cbd, KD, "cb")
        bv_sb = load_bias(bvd, KD, "bv")

        ones8 = small.tile([P, 2, P], F8, tag="ones", name="ones")
        nc.vector.memset(ones8, 1.0)

        def alloc_pairs(pool, pairs, n, tag, dt=F8):
            """fp8 pair-packed tiles [P, 2, n]."""
            return [pool.tile([P, 2, n], dt, tag=f"{tag}{t}", name=f"{tag}{t}")
                    for t in range(pairs)]

        def load_pairs(tiles, src, n):
            for t, tl in enumerate(tiles):
                nc.sync.dma_start(
                    out=tl,
                    in_=src[t * P:(t + 1) * P, :].rearrange(
                        "p (r s) -> p r s", r=2))

        def fm_layer8(psum, src8, w8, pairs, mtiles, bias_sb, func, dst8,
                      tb_outer=False):
            """fp8 DoubleRow feature-major layer into pair-packed fp8 dst.

            tb_outer runs token blocks in the outer loop so each block only
            needs 1/NBLK of src8 -- used for L1 whose src is still in
            flight from DRAM when compute starts."""
            outer, inner = ((NBLK, mtiles) if tb_outer else (mtiles, NBLK))
            for o in range(outer):
                pss = [psum.tile([P, NB], F32, tag="mm", name="mm")
                       for _ in range(inner)]
                for t in range(pairs):
                    for i in range(inner):
                        m, tb = (i, o) if tb_outer else (o, i)
                        nc.tensor.matmul(
                            pss[i], w8[t][:, :, m * P:(m + 1) * P],
                            src8[t][:, :, tb * NB:(tb + 1) * NB],
                            start=(t == 0), stop=(t == pairs - 1),
                            perf_mode=DR)
                for i in range(inner):
                    m, tb = (i, o) if tb_outer else (o, i)
                    dst = dst8[m // 2][:, m % 2, tb * NB:(tb + 1) * NB]
                    nc.scalar.activation(
                        dst, pss[i], func,
                        bias=0.0 if bias_sb is None else bias_sb[:, m:m + 1],
                        scale=1.0)

        # ------ persistent attention operands + y prefetch ------
        with tc.tile_pool(name="pz", bufs=1) as pz, \
             tc.tile_pool(name="py", bufs=1) as py, \
             tc.tile_pool(name="pyv", bufs=1) as pyv, \
             tc.tile_pool(name="pwv", bufs=1) as pwv:
            z8 = alloc_pairs(pz, PD, S, "z8")
            y8 = alloc_pairs(py, PD, S, "y8")
            yv8t = alloc_pairs(pyv, NKV2, D, "yv8t")
            wv8 = alloc_pairs(pwv, PD, D, "wv8")

            # ---------------- Stage A: x-MLP -> z8 (in SBUF) ----------------
            with tc.tile_pool(name="wx", bufs=1) as wx, \
                 tc.tile_pool(name="px", bufs=1) as px, \
                 tc.tile_pool(name="phA", bufs=1) as phA, \
                 tc.tile_pool(name="phB", bufs=1) as phB, \
                 tc.tile_pool(name="psA", bufs=8, space="PSUM") as psA:
                x8 = alloc_pairs(px, PD, S, "x8")
                w18 = alloc_pairs(wx, PD, D, "w18")
                # L1 runs token-block-outer: block tb only needs x8[*][tb],
                # so land full W1 + the first x8 block first, then stream
                # the remaining x8 blocks ahead of their use.
                x8r = x8d.rearrange("k (r s) -> k r s", r=2)
                for t in range(PD):
                    sl = slice(t * P, (t + 1) * P)
                    nc.sync.dma_start(
                        out=w18[t], in_=W1d[sl, :].rearrange(
                            "p (r s) -> p r s", r=2))
                    nc.sync.dma_start(out=x8[t][:, :, 0:NB],
                                      in_=x8r[sl, :, 0:NB])
                for tb in range(1, NBLK):
                    for t in range(PD):
                        sl = slice(t * P, (t + 1) * P)
                        nc.sync.dma_start(
                            out=x8[t][:, :, tb * NB:(tb + 1) * NB],
                            in_=x8r[sl, :, tb * NB:(tb + 1) * NB])
                w28 = alloc_pairs(wx, PD, DB, "w28")
                load_pairs(w28, W2d, DB)
                w38 = alloc_pairs(wx, PB, D, "w38")
                load_pairs(w38, W3d, D)
                w48 = alloc_pairs(wx, PD, D, "w48")
                load_pairs(w48, W4d, D)
                a8 = alloc_pairs(wx, PD, D, "a8")
                load_pairs(a8, A8d, D)
                # y-side prefetch (queued behind stage A's needs)
                load_pairs(y8, y8d, S)
                load_pairs(wv8, Wvd, D)

                h18 = alloc_pairs(phA, PD, S, "ha")
                h28 = alloc_pairs(phB, PB, S, "hb")
                h38 = alloc_pairs(phA, PD, S, "ha")   # reuse phA slots
                h48 = alloc_pairs(phB, PD, S, "hb")   # grow phB to 4 pair slots
                fm_layer8(psA, x8, w18, PD, KD, b1_sb, AF.Relu, h18,
                          tb_outer=True)
                fm_layer8(psA, h18, w28, PD, KB, b2_sb, AF.Relu, h28)
                fm_layer8(psA, h28, w38, PB, KD, b3_sb, AF.Relu, h38)
                fm_layer8(psA, h38, w48, PD, KD, b4_sb, AF.Relu, h48)
                fm_layer8(psA, h48, a8, PD, KD, None, AF.Identity, z8)

            # ------------ Stage B: yv = y@Wv (fp8, requant to kv-pairs) -----
            with tc.tile_pool(name="psBv", bufs=4, space="PSUM") as psBv:
                for tkv in range(NT):
                    for db in range(2):
                        pv = psBv.tile([P, NB], F32, tag="vv", name="vv")
                        for t in range(PD):
                            nc.tensor.matmul(
                                pv, y8[t][:, :, tkv * P:(tkv + 1) * P],
                                wv8[t][:, :, db * NB:(db + 1) * NB],
                                start=(t == 0), stop=(t == PD - 1),
                                perf_mode=DR)
                        nc.vector.tensor_copy(
                            out=yv8t[tkv // 2][:, tkv % 2,
                                               db * NB:(db + 1) * NB],
                            in_=pv)

            # ---------------- Stage C: attention (single pass) ----------------
            with tc.tile_pool(name="pE", bufs=1) as pE, \
                 tc.tile_pool(name="pT", bufs=10) as pT, \
                 tc.tile_pool(name="psCs", bufs=5, space="PSUM") as psCs, \
                 tc.tile_pool(name="psEY", bufs=2, space="PSUM") as psEY, \
                 tc.tile_pool(name="psRS", bufs=1, space="PSUM") as psRS:
                et1 = alloc_pairs(pE, NKV2, S, "e")
                # scores^T -> E-1 in fp8, kv pair-packed, per q block
                for qb in range(NBLK):
                    for tk in range(NT):
                        ps = psCs.tile([P, NB], F32, tag="sc", name="sc")
                        for t in range(PD):
                            nc.tensor.matmul(
                                ps, y8[t][:, :, tk * P:(tk + 1) * P],
                                z8[t][:, :, qb * NB:(qb + 1) * NB],
                                start=(t == 0), stop=(t == PD - 1),
                                perf_mode=DR)
                        etmp = pT.tile([P, NB], BF, tag="et", name="et")
                        nc.scalar.activation(etmp, ps, AF.Exp,
                                             bias=wb_sb[:, tk:tk + 1],
                                             scale=SCALE)
                        nc.vector.tensor_scalar_add(
                            et1[tk // 2][:, tk % 2, qb * NB:(qb + 1) * NB],
                            etmp, -1.0)
                for qb in range(NBLK):
                    # rowsum(E) = 2048 + sum(E-1), replicated on all
                    # partitions via the all-ones stationary
                    prs = psRS.tile([P, NB], F32, tag="rs", name="rs")
                    for t2 in range(NKV2):
                        nc.tensor.matmul(
                            prs, ones8,
                            et1[t2][:, :, qb * NB:(qb + 1) * NB],
                            start=(t2 == 0), stop=(t2 == NKV2 - 1),
                            perf_mode=DR)
                    # rowsum = S + s with |s| < ~4, so one Newton step from
                    # 1/S is exact to ~2e-6: rinv = 1/S - s/S^2. Replaces a
                    # 3.3us DVE reciprocal with one cheap tensor_scalar.
                    rrep = rpool.tile([P, NB], F32, tag=f"rp{qb % 2}",
                                      name=f"rp{qb % 2}")
                    nc.vector.tensor_scalar(rrep, prs, -1.0 / (S * S),
                                            1.0 / S, op0=MULT, op1=ADD)
                    # out^T = ((E-1)@yv + c) * rinv + bv
                    for dt_ in range(KD):
                        po = psEY.tile([P, NB], F32, tag="oo", name="oo")
                        for t2 in range(NKV2):
                            nc.tensor.matmul(
                                po, yv8t[t2][:, :, dt_ * P:(dt_ + 1) * P],
                                et1[t2][:, :, qb * NB:(qb + 1) * NB],
                                start=(t2 == 0), stop=(t2 == NKV2 - 1),
                                perf_mode=DR)
                        tmp = pT.tile([P, NB], F32, tag="tm", name="tm")
                        nc.vector.scalar_tensor_tensor(
                            tmp, po, cb_sb[:, dt_:dt_ + 1], rrep,
                            op0=ADD, op1=MULT)
                        ot = outp.tile([P, NB], F32, tag="ot", name="ot")
                        nc.scalar.activation(ot, tmp, AF.Identity,
                                             bias=bv_sb[:, dt_:dt_ + 1],
                                             scale=1.0)
                        nc.sync.dma_start(
                            out=outT[dt_ * P:(dt_ + 1) * P,
                                     qb * NB:(qb + 1) * NB],
                            in_=ot)

    nc.compile()
    _NC = nc
    return nc


def _pack8(w):
    """[K, N] -> DoubleRow pair-packed fp8 [K/2, 2N]:
    out[t*128+p, r*N+m] = w[(2t+r)*128+p, m]."""
    K, N = w.shape
    return np.ascontiguousarray(
        w.astype(FP8).reshape(K // 256, 2, 128, N)
        .transpose(0, 2, 1, 3).reshape(K // 2, 2 * N))


def make_in_maps(inputs):
    """Host-side prep: per-core batch shard, fp8 casts + pair packing,
    feature-major transposes of x/y, weight folding A = Wq@Wk^T, and the
    exact O(S*D) correction vectors w = SCALE*(y@(Wk@bq)), c = colsum(y)@Wv.
    """
    x = np.asarray(inputs["x"])
    y = np.asarray(inputs["y"])
    Wq = np.asarray(inputs["Wq"]).astype(np.float64)
    Wk = np.asarray(inputs["Wk"]).astype(np.float64)
    Wv = np.asarray(inputs["Wv"]).astype(np.float64)
    bq = np.asarray(inputs["bq"]).astype(np.float64)
    A = (Wq @ Wk.T).astype(np.float32)
    wk_bq = Wk @ bq
    shared = {}
    for k in ("W1", "W2", "W3", "W4"):
        shared[k] = _pack8(np.asarray(inputs[k]).astype(np.float32))
    shared["A8"] = _pack8(A)
    shared["Wv8"] = _pack8(np.asarray(inputs["Wv"]).astype(np.float32))
    for k, nt in (("b1", KD), ("b2", KB), ("b3", KD), ("b4", KD)):
        shared[k] = np.ascontiguousarray(
            np.asarray(inputs[k]).astype(np.float32).reshape(nt, P).T)
    shared["bvb"] = np.ascontiguousarray(
        np.asarray(inputs["bv"]).astype(np.float32).reshape(KD, P).T)
    in_maps = []
    for b in range(x.shape[0]):
        m = dict(shared)
        m["x8"] = _pack8(np.ascontiguousarray(x[b].T))
        m["y8"] = _pack8(np.ascontiguousarray(y[b].T))
        yb = y[b].astype(np.float64)
        m["wb"] = np.ascontiguousarray(
            (SCALE * (yb @ wk_bq)).astype(np.float32).reshape(NT, P).T)
        m["cb"] = np.ascontiguousarray(
            (yb.sum(0) @ Wv).astype(np.float32).reshape(KD, P).T)
        in_maps.append(m)
    return in_maps


def kernel(**inputs):
    from concourse.bass_utils import run_bass_kernel_spmd

    nc = build_nc()
    in_maps = make_in_maps(inputs)
    res = run_bass_kernel_spmd(nc, in_maps, list(range(len(in_maps))))
    return np.stack([np.asarray(r["outT"], dtype=np.float32).T
                     for r in res.results])


# revision 21
# speedup vs baseline: 1.0295x; 1.0112x over previous
"""Trainium2 Bass kernel for nn_CrossAttention (4-layer MLP -> cross-attention).

Sharding: data-parallel across batch B=8, one batch element per NeuronCore.

All matmuls run in fp8(e4m3) DoubleRow (2 contraction rows per PE pass -> 2x
rate, 157 TF/s). Three algebraic folds cut the per-core matmul work from the
naive 43 GFLOP to ~31 GFLOP-equivalent:

1. Scores fold: s = q@k^T with q = h@Wq+bq, k = y@Wk+bk expands to
   h@(Wq@Wk^T)@y^T + u[q] + w[kv] + const. The per-q terms drop out of
   softmax (shift invariance along kv), so with A = Wq@Wk^T (host-folded
   weights) and w = y@(Wk@bq) (exact, host, O(S*D)) the k-projection
   disappears: z = h@A, s_eff^T = z@y^T + w[kv], w folded into the exp bias
   (per-partition in the transposed layout).

2. Value fold: out = softmax@(y@Wv+bv) needs E@(y@Wv). yv = y@Wv is
   computed once on device (fp8, requantized to fp8), then the shift trick
       E@yv = (E-1)@yv + ones (x) colsum(y@Wv)
   keeps fp8 viable: (E-1) ~ +-0.1 (scores are small) so it quantizes to
   fp8 with ~4e-3 abs error, while E itself (~1.0) would not.
   c = colsum(y)@Wv is computed on host in fp64 (O(S*D)); using the EXACT
   c (not colsum of quantized yv) cancels the correlated fp8-quantization
   bias of yv to first order (the residual is (E-1)-weighted).

3. rowsum(E) = 2048 + sum(E-1) via an all-ones [128,2,128] fp8 stationary
   matmul whose output is the rowsum replicated across all 128 partitions
   (no partition broadcast needed).

   out^T[d,q] = ((E-1)@yv)^T[d,q] + c[d]) * rinv[q] + bv[d]; output is
   stored transposed [D,S] and untransposed on host.

Layout: the MLP runs feature-major (h^T = W^T @ h^T, no transposes); scores
come out transposed (kv on partitions) and feed (E-1)@yv directly as the
moving operand against token-pair-packed yv; the result is the final out^T.
No PE transposes anywhere.

fp8 operands are pair-packed for DoubleRow: logical contraction index
k = (2t+r)*128+p lives in tile t, partition p, middle index r, i.e. SBUF
tiles [128, 2, N] (packed on host to [K/2, 2*N] so each tile is one DMA).

Engine split: scalar = relu/exp psum drains; vector = yv requant,
(E-1)->fp8, rowsum fixup, reciprocal, (x+c)*rinv; gpsimd = +bv.
All accumulation fp32 in PSUM. Fully SBUF-resident.
"""

import sys

if "/opt/trn_rl_repo" not in sys.path:
    sys.path.insert(0, "/opt/trn_rl_repo")

import numpy as np
import ml_dtypes

P = 128
D = 1024
DB = 512
S = 2048
KD = D // P       # 8 feature tiles of 128
KB = DB // P      # 4
PD = KD // 2      # 4 fp8 pair-tiles for a 1024 contraction
PB = KB // 2      # 2 for 512
NT = S // P       # 16 token tiles
NKV2 = NT // 2    # 8 token pair-tiles for the 2048 kv contraction
NB = 512          # moving-operand free-dim block
NBLK = S // NB    # 4 token blocks
HALF = S // 2     # q processed in 2 halves during attention
QB = HALF // NB   # 2 q blocks per half
NCORES = 8
SCALE = float(1.0 / np.sqrt(D))

BF16 = ml_dtypes.bfloat16
FP8 = ml_dtypes.float8_e4m3

_NC = None


def build_nc():
    """Build + compile the per-core Bass program (cached)."""
    global _NC
    if _NC is not None:
        return _NC

    from contextlib import ExitStack
    import concourse.bass as bass
    import concourse.tile as tile
    from concourse import bacc, mybir

    BF = mybir.dt.bfloat16
    F8 = mybir.dt.float8e4
    F32 = mybir.dt.float32
    AF = mybir.ActivationFunctionType
    DR = mybir.MatmulPerfMode.DoubleRow
    ADD = mybir.AluOpType.add
    MULT = mybir.AluOpType.mult

    nc = bacc.Bacc("TRN2", target_bir_lowering=False, debug=False,
                   num_devices=NCORES)

    def din(name, shape, dt):
        return nc.dram_tensor(name, shape, dt, kind="ExternalInput").ap()

    # fp8 operands arrive pair-packed: [K/2, 2*N]
    x8d = din("x8", [D // 2, 2 * S], F8)
    y8d = din("y8", [D // 2, 2 * S], F8)      # feature-major
    W1d = din("W1", [D // 2, 2 * D], F8)
    W2d = din("W2", [D // 2, 2 * DB], F8)
    W3d = din("W3", [DB // 2, 2 * D], F8)
    W4d = din("W4", [D // 2, 2 * D], F8)
    A8d = din("A8", [D // 2, 2 * D], F8)      # Wq @ Wk^T, host-folded
    Wvd = din("Wv8", [D // 2, 2 * D], F8)
    b1 = din("b1", [P, KD], F32)
    b2 = din("b2", [P, KB], F32)
    b3 = din("b3", [P, KD], F32)
    b4 = din("b4", [P, KD], F32)
    wbd = din("wb", [P, NT], F32)    # SCALE * (y @ (Wk@bq)), kv-major cols
    cbd = din("cb", [P, KD], F32)    # (colsum(y)@Wv)[d], feature-major cols
    bvd = din("bvb", [P, KD], F32)   # bv[d], feature-major cols
    outT = nc.dram_tensor("outT", [D, S], F32, kind="ExternalOutput").ap()

    with tile.TileContext(nc) as tc, ExitStack() as ctx:
        small = ctx.enter_context(tc.tile_pool(name="small", bufs=1))
        rpool = ctx.enter_context(tc.tile_pool(name="rpool", bufs=4))
        outp = ctx.enter_context(tc.tile_pool(name="outp", bufs=8))

        def load_bias(src, cols, tag):
            t = small.tile([P, cols], F32, tag=tag, name=tag)
            nc.gpsimd.dma_start(out=t, in_=src)
            return t

        b1_sb = load_bias(b1, KD, "b1")
        b2_sb = load_bias(b2, KB, "b2")
        b3_sb = load_bias(b3, KD, "b3")
        b4_sb = load_bias(b4, KD, "b4")
        wb_sb = load_bias(wbd, NT, "wb")
        cb_sb = load_bias(cbd, KD, "cb")
        bv_sb = load_bias(bvd, KD, "bv")

        ones8 = small.tile([P, 2, P], F8, tag="ones", name="ones")
        nc.vector.memset(ones8, 1.0)

        def alloc_pairs(pool, pairs, n, tag, dt=F8):
            """fp8 pair-packed tiles [P, 2, n]."""
            return [pool.tile([P, 2, n], dt, tag=f"{tag}{t}", name=f"{tag}{t}")
                    for t in range(pairs)]

        def load_pairs(tiles, src, n):
            for t, tl in enumerate(tiles):
                nc.sync.dma_start(
                    out=tl,
                    in_=src[t * P:(t + 1) * P, :].rearrange(
                        "p (r s) -> p r s", r=2))

        def fm_layer8(psum, src8, w8, pairs, mtiles, bias_sb, func, dst8,
                      tb_outer=False):
            """fp8 DoubleRow feature-major layer into pair-packed fp8 dst.

            tb_outer runs token blocks in the outer loop so each block only
            needs 1/NBLK of src8 -- used for L1 whose src is still in
            flight from DRAM when compute starts."""
            outer, inner = ((NBLK, mtiles) if tb_outer else (mtiles, NBLK))
            for o in range(outer):
                pss = [psum.tile([P, NB], F32, tag="mm", name="mm")
                       for _ in range(inner)]
                for t in range(pairs):
                    for i in range(inner):
                        m, tb = (i, o) if tb_outer else (o, i)
                        nc.tensor.matmul(
                            pss[i], w8[t][:, :, m * P:(m + 1) * P],
                            src8[t][:, :, tb * NB:(tb + 1) * NB],
                            start=(t == 0), stop=(t == pairs - 1),
                            perf_mode=DR)
                for i in range(inner):
                    m, tb = (i, o) if tb_outer else (o, i)
                    dst = dst8[m // 2][:, m % 2, tb * NB:(tb + 1) * NB]
                    nc.scalar.activation(
                        dst, pss[i], func,
                        bias=0.0 if bias_sb is None else bias_sb[:, m:m + 1],
                        scale=1.0)

        # ------ persistent attention operands + y prefetch ------
        with tc.tile_pool(name="pz", bufs=1) as pz, \
             tc.tile_pool(name="py", bufs=1) as py, \
             tc.tile_pool(name="pyv", bufs=1) as pyv, \
             tc.tile_pool(name="pwv", bufs=1) as pwv:
            z8 = alloc_pairs(pz, PD, S, "z8")
            y8 = alloc_pairs(py, PD, S, "y8")
            yv8t = alloc_pairs(pyv, NKV2, D, "yv8t")
            wv8 = alloc_pairs(pwv, PD, D, "wv8")

            # ---------------- Stage A: x-MLP -> z8 (in SBUF) ----------------
            with tc.tile_pool(name="wx", bufs=1) as wx, \
                 tc.tile_pool(name="px", bufs=1) as px, \
                 tc.tile_pool(name="phA", bufs=1) as phA, \
                 tc.tile_pool(name="phB", bufs=1) as phB, \
                 tc.tile_pool(name="psA", bufs=8, space="PSUM") as psA:
                x8 = alloc_pairs(px, PD, S, "x8")
                w18 = alloc_pairs(wx, PD, D, "w18")
                # L1 runs token-block-outer: block tb only needs x8[*][tb],
                # so land full W1 + the first x8 block first, then stream
                # the remaining x8 blocks ahead of their use.
                x8r = x8d.rearrange("k (r s) -> k r s", r=2)
                for t in range(PD):
                    sl = slice(t * P, (t + 1) * P)
                    nc.sync.dma_start(
                        out=w18[t], in_=W1d[sl, :].rearrange(
                            "p (r s) -> p r s", r=2))
                    nc.sync.dma_start(out=x8[t][:, :, 0:NB],
                                      in_=x8r[sl, :, 0:NB])
                for tb in range(1, NBLK):
                    for t in range(PD):
                        sl = slice(t * P, (t + 1) * P)
                        nc.sync.dma_start(
                            out=x8[t][:, :, tb * NB:(tb + 1) * NB],
                            in_=x8r[sl, :, tb * NB:(tb + 1) * NB])
                w28 = alloc_pairs(wx, PD, DB, "w28")
                load_pairs(w28, W2d, DB)
                w38 = alloc_pairs(wx, PB, D, "w38")
                load_pairs(w38, W3d, D)
                w48 = alloc_pairs(wx, PD, D, "w48")
                load_pairs(w48, W4d, D)
                a8 = alloc_pairs(wx, PD, D, "a8")
                load_pairs(a8, A8d, D)
                # y-side prefetch (queued behind stage A's needs)
                load_pairs(y8, y8d, S)
                load_pairs(wv8, Wvd, D)

                h18 = alloc_pairs(phA, PD, S, "ha")
                h28 = alloc_pairs(phB, PB, S, "hb")
                h38 = alloc_pairs(phA, PD, S, "ha")   # reuse phA slots
                h48 = alloc_pairs(phB, PD, S, "hb")   # grow phB to 4 pair slots
                fm_layer8(psA, x8, w18, PD, KD, b1_sb, AF.Relu, h18,
                          tb_outer=True)
                fm_layer8(psA, h18, w28, PD, KB, b2_sb, AF.Relu, h28)
                fm_layer8(psA, h28, w38, PB, KD, b3_sb, AF.Relu, h38)
                fm_layer8(psA, h38, w48, PD, KD, b4_sb, AF.Relu, h48)
                fm_layer8(psA, h48, a8, PD, KD, None, AF.Identity, z8)

                # yv = y@Wv (fp8, requant to kv-pairs) -- same psum rotation
                # as the MLP (same pool+tag) so there is no bank-handoff
                # bubble at the z->yv phase boundary.
                for tkv in range(NT):
                    for db in range(2):
                        pv = psA.tile([P, NB], F32, tag="mm", name="mm")
                        for t in range(PD):
                            nc.tensor.matmul(
                                pv, y8[t][:, :, tkv * P:(tkv + 1) * P],
                                wv8[t][:, :, db * NB:(db + 1) * NB],
                                start=(t == 0), stop=(t == PD - 1),
                                perf_mode=DR)
                        nc.vector.tensor_copy(
                            out=yv8t[tkv // 2][:, tkv % 2,
                                               db * NB:(db + 1) * NB],
                            in_=pv)

            # ---------------- Stage C: attention (single pass) ----------------
            with tc.tile_pool(name="pE", bufs=1) as pE, \
                 tc.tile_pool(name="pT", bufs=10) as pT, \
                 tc.tile_pool(name="psCs", bufs=5, space="PSUM") as psCs, \
                 tc.tile_pool(name="psEY", bufs=2, space="PSUM") as psEY, \
                 tc.tile_pool(name="psRS", bufs=1, space="PSUM") as psRS:
                et1 = alloc_pairs(pE, NKV2, S, "e")
                # scores^T -> E-1 in fp8, kv pair-packed, per q block
                for qb in range(NBLK):
                    for tk in range(NT):
                        ps = psCs.tile([P, NB], F32, tag="sc", name="sc")
                        for t in range(PD):
                            nc.tensor.matmul(
                                ps, y8[t][:, :, tk * P:(tk + 1) * P],
                                z8[t][:, :, qb * NB:(qb + 1) * NB],
                                start=(t == 0), stop=(t == PD - 1),
                                perf_mode=DR)
                        etmp = pT.tile([P, NB], BF, tag="et", name="et")
                        nc.scalar.activation(etmp, ps, AF.Exp,
                                             bias=wb_sb[:, tk:tk + 1],
                                             scale=SCALE)
                        nc.vector.tensor_scalar_add(
                            et1[tk // 2][:, tk % 2, qb * NB:(qb + 1) * NB],
                            etmp, -1.0)
                for qb in range(NBLK):
                    # rowsum(E) = 2048 + sum(E-1), replicated on all
                    # partitions via the all-ones stationary
                    prs = psRS.tile([P, NB], F32, tag="rs", name="rs")
                    for t2 in range(NKV2):
                        nc.tensor.matmul(
                            prs, ones8,
                            et1[t2][:, :, qb * NB:(qb + 1) * NB],
                            start=(t2 == 0), stop=(t2 == NKV2 - 1),
                            perf_mode=DR)
                    # rowsum = S + s with |s| < ~4, so one Newton step from
                    # 1/S is exact to ~2e-6: rinv = 1/S - s/S^2. Replaces a
                    # 3.3us DVE reciprocal with one cheap tensor_scalar.
                    rrep = rpool.tile([P, NB], F32, tag=f"rp{qb % 2}",
                                      name=f"rp{qb % 2}")
                    nc.vector.tensor_scalar(rrep, prs, -1.0 / (S * S),
                                            1.0 / S, op0=MULT, op1=ADD)
                    # out^T = ((E-1)@yv + c) * rinv + bv
                    for dt_ in range(KD):
                        po = psEY.tile([P, NB], F32, tag="oo", name="oo")
                        for t2 in range(NKV2):
                            nc.tensor.matmul(
                                po, yv8t[t2][:, :, dt_ * P:(dt_ + 1) * P],
                                et1[t2][:, :, qb * NB:(qb + 1) * NB],
                                start=(t2 == 0), stop=(t2 == NKV2 - 1),
                                perf_mode=DR)
                        tmp = pT.tile([P, NB], F32, tag="tm", name="tm")
                        nc.vector.scalar_tensor_tensor(
                            tmp, po, cb_sb[:, dt_:dt_ + 1], rrep,
                            op0=ADD, op1=MULT)
                        ot = outp.tile([P, NB], F32, tag="ot", name="ot")
                        nc.scalar.activation(ot, tmp, AF.Identity,
                                             bias=bv_sb[:, dt_:dt_ + 1],
                                             scale=1.0)
                        nc.sync.dma_start(
                            out=outT[dt_ * P:(dt_ + 1) * P,
                                     qb * NB:(qb + 1) * NB],
                            in_=ot)

    nc.compile()
    _NC = nc
    return nc


def _pack8(w):
    """[K, N] -> DoubleRow pair-packed fp8 [K/2, 2N]:
    out[t*128+p, r*N+m] = w[(2t+r)*128+p, m]."""
    K, N = w.shape
    return np.ascontiguousarray(
        w.astype(FP8).reshape(K // 256, 2, 128, N)
        .transpose(0, 2, 1, 3).reshape(K // 2, 2 * N))


def make_in_maps(inputs):
    """Host-side prep: per-core batch shard, fp8 casts + pair packing,
    feature-major transposes of x/y, weight folding A = Wq@Wk^T, and the
    exact O(S*D) correction vectors w = SCALE*(y@(Wk@bq)), c = colsum(y)@Wv.
    """
    x = np.asarray(inputs["x"])
    y = np.asarray(inputs["y"])
    Wq = np.asarray(inputs["Wq"]).astype(np.float64)
    Wk = np.asarray(inputs["Wk"]).astype(np.float64)
    Wv = np.asarray(inputs["Wv"]).astype(np.float64)
    bq = np.asarray(inputs["bq"]).astype(np.float64)
    A = (Wq @ Wk.T).astype(np.float32)
    wk_bq = Wk @ bq
    shared = {}
    for k in ("W1", "W2", "W3", "W4"):
        shared[k] = _pack8(np.asarray(inputs[k]).astype(np.float32))
    shared["A8"] = _pack8(A)
    shared["Wv8"] = _pack8(np.asarray(inputs["Wv"]).astype(np.float32))
    for k, nt in (("b1", KD), ("b2", KB), ("b3", KD), ("b4", KD)):
        shared[k] = np.ascontiguousarray(
            np.asarray(inputs[k]).astype(np.float32).reshape(nt, P).T)
    shared["bvb"] = np.ascontiguousarray(
        np.asarray(inputs["bv"]).astype(np.float32).reshape(KD, P).T)
    in_maps = []
    for b in range(x.shape[0]):
        m = dict(shared)
        m["x8"] = _pack8(np.ascontiguousarray(x[b].T))
        m["y8"] = _pack8(np.ascontiguousarray(y[b].T))
        yb = y[b].astype(np.float64)
        m["wb"] = np.ascontiguousarray(
            (SCALE * (yb @ wk_bq)).astype(np.float32).reshape(NT, P).T)
        m["cb"] = np.ascontiguousarray(
            (yb.sum(0) @ Wv).astype(np.float32).reshape(KD, P).T)
        in_maps.append(m)
    return in_maps


def kernel(**inputs):
    from concourse.bass_utils import run_bass_kernel_spmd

    nc = build_nc()
    in_maps = make_in_maps(inputs)
    res = run_bass_kernel_spmd(nc, in_maps, list(range(len(in_maps))))
    return np.stack([np.asarray(r["outT"], dtype=np.float32).T
                     for r in res.results])


# revision 27
# speedup vs baseline: 1.0357x; 1.0060x over previous
"""Trainium2 Bass kernel for nn_CrossAttention (4-layer MLP -> cross-attention).

Sharding: data-parallel across batch B=8, one batch element per NeuronCore.

All matmuls run in fp8(e4m3) DoubleRow (2 contraction rows per PE pass -> 2x
rate, 157 TF/s). Three algebraic folds cut the per-core matmul work from the
naive 43 GFLOP to ~31 GFLOP-equivalent:

1. Scores fold: s = q@k^T with q = h@Wq+bq, k = y@Wk+bk expands to
   h@(Wq@Wk^T)@y^T + u[q] + w[kv] + const. The per-q terms drop out of
   softmax (shift invariance along kv), so with A = Wq@Wk^T (host-folded
   weights) and w = y@(Wk@bq) (exact, host, O(S*D)) the k-projection
   disappears: z = h@A, s_eff^T = z@y^T + w[kv], w folded into the exp bias
   (per-partition in the transposed layout).

2. Value fold: out = softmax@(y@Wv+bv) needs E@(y@Wv). yv = y@Wv is
   computed once on device (fp8, requantized to fp8), then the shift trick
       E@yv = (E-1)@yv + ones (x) colsum(y@Wv)
   keeps fp8 viable: (E-1) ~ +-0.1 (scores are small) so it quantizes to
   fp8 with ~4e-3 abs error, while E itself (~1.0) would not.
   c = colsum(y)@Wv is computed on host in fp64 (O(S*D)); using the EXACT
   c (not colsum of quantized yv) cancels the correlated fp8-quantization
   bias of yv to first order (the residual is (E-1)-weighted).

3. rowsum(E) = 2048 + sum(E-1) via an all-ones [128,2,128] fp8 stationary
   matmul whose output is the rowsum replicated across all 128 partitions
   (no partition broadcast needed).

   out^T[d,q] = ((E-1)@yv)^T[d,q] + c[d]) * rinv[q] + bv[d]; output is
   stored transposed [D,S] and untransposed on host.

Layout: the MLP runs feature-major (h^T = W^T @ h^T, no transposes); scores
come out transposed (kv on partitions) and feed (E-1)@yv directly as the
moving operand against token-pair-packed yv; the result is the final out^T.
No PE transposes anywhere.

fp8 operands are pair-packed for DoubleRow: logical contraction index
k = (2t+r)*128+p lives in tile t, partition p, middle index r, i.e. SBUF
tiles [128, 2, N] (packed on host to [K/2, 2*N] so each tile is one DMA).

Engine split: scalar = relu/exp psum drains; vector = yv requant,
(E-1)->fp8, rowsum fixup, reciprocal, (x+c)*rinv; gpsimd = +bv.
All accumulation fp32 in PSUM. Fully SBUF-resident.
"""

import sys

if "/opt/trn_rl_repo" not in sys.path:
    sys.path.insert(0, "/opt/trn_rl_repo")

import numpy as np
import ml_dtypes

P = 128
D = 1024
DB = 512
S = 2048
KD = D // P       # 8 feature tiles of 128
KB = DB // P      # 4
PD = KD // 2      # 4 fp8 pair-tiles for a 1024 contraction
PB = KB // 2      # 2 for 512
NT = S // P       # 16 token tiles
NKV2 = NT // 2    # 8 token pair-tiles for the 2048 kv contraction
NB = 512          # moving-operand free-dim block
NBLK = S // NB    # 4 token blocks
HALF = S // 2     # q processed in 2 halves during attention
QB = HALF // NB   # 2 q blocks per half
NCORES = 8
SCALE = float(1.0 / np.sqrt(D))

BF16 = ml_dtypes.bfloat16
FP8 = ml_dtypes.float8_e4m3

_NC = None


def build_nc():
    """Build + compile the per-core Bass program (cached)."""
    global _NC
    if _NC is not None:
        return _NC

    from contextlib import ExitStack
    import concourse.bass as bass
    import concourse.tile as tile
    from concourse import bacc, mybir

    BF = mybir.dt.bfloat16
    F8 = mybir.dt.float8e4
    F32 = mybir.dt.float32
    AF = mybir.ActivationFunctionType
    DR = mybir.MatmulPerfMode.DoubleRow
    ADD = mybir.AluOpType.add
    MULT = mybir.AluOpType.mult

    nc = bacc.Bacc("TRN2", target_bir_lowering=False, debug=False,
                   num_devices=NCORES)

    def din(name, shape, dt):
        return nc.dram_tensor(name, shape, dt, kind="ExternalInput").ap()

    # fp8 operands arrive pair-packed: [K/2, 2*N]
    x8d = din("x8", [D // 2, 2 * S], F8)
    y8d = din("y8", [D // 2, 2 * S], F8)      # feature-major
    W1d = din("W1", [D // 2, 2 * D], F8)
    W2d = din("W2", [D // 2, 2 * DB], F8)
    W3d = din("W3", [DB // 2, 2 * D], F8)
    W4d = din("W4", [D // 2, 2 * D], F8)
    A8d = din("A8", [D // 2, 2 * D], F8)      # Wq @ Wk^T, host-folded
    Wvd = din("Wv8", [D // 2, 2 * D], F8)
    b1 = din("b1", [P, KD], F32)
    b2 = din("b2", [P, KB], F32)
    b3 = din("b3", [P, KD], F32)
    b4 = din("b4", [P, KD], F32)
    wbd = din("wb", [P, NT], F32)    # SCALE * (y @ (Wk@bq)), kv-major cols
    cbd = din("cb", [P, KD], F32)    # (colsum(y)@Wv)[d], feature-major cols
    bvd = din("bvb", [P, KD], F32)   # bv[d], feature-major cols
    outT = nc.dram_tensor("outT", [D, S], F32, kind="ExternalOutput").ap()

    with tile.TileContext(nc) as tc, ExitStack() as ctx:
        small = ctx.enter_context(tc.tile_pool(name="small", bufs=1))
        rpool = ctx.enter_context(tc.tile_pool(name="rpool", bufs=4))
        outp = ctx.enter_context(tc.tile_pool(name="outp", bufs=8))
        # one 8-bank psum rotation shared by every [P,NB] accumulator in the
        # kernel -- no pool handoff bubbles at phase boundaries
        psA = ctx.enter_context(tc.tile_pool(name="psA", bufs=8,
                                             space="PSUM"))

        def load_bias(src, cols, tag):
            t = small.tile([P, cols], F32, tag=tag, name=tag)
            nc.gpsimd.dma_start(out=t, in_=src)
            return t

        b1_sb = load_bias(b1, KD, "b1")
        b2_sb = load_bias(b2, KB, "b2")
        b3_sb = load_bias(b3, KD, "b3")
        b4_sb = load_bias(b4, KD, "b4")
        wb_sb = load_bias(wbd, NT, "wb")
        cb_sb = load_bias(cbd, KD, "cb")
        bv_sb = load_bias(bvd, KD, "bv")

        ones8 = small.tile([P, 2, P], F8, tag="ones", name="ones")
        nc.vector.memset(ones8, 1.0)

        def alloc_pairs(pool, pairs, n, tag, dt=F8):
            """fp8 pair-packed tiles [P, 2, n]."""
            return [pool.tile([P, 2, n], dt, tag=f"{tag}{t}", name=f"{tag}{t}")
                    for t in range(pairs)]

        def load_pairs(tiles, src, n):
            for t, tl in enumerate(tiles):
                nc.sync.dma_start(
                    out=tl,
                    in_=src[t * P:(t + 1) * P, :].rearrange(
                        "p (r s) -> p r s", r=2))

        def fm_layer8(psum, src8, w8, pairs, mtiles, bias_sb, func, dst8,
                      tb_outer=False):
            """fp8 DoubleRow feature-major layer into pair-packed fp8 dst.

            tb_outer runs token blocks in the outer loop so each block only
            needs 1/NBLK of src8 -- used for L1 whose src is still in
            flight from DRAM when compute starts."""
            outer, inner = ((NBLK, mtiles) if tb_outer else (mtiles, NBLK))
            for o in range(outer):
                pss = [psum.tile([P, NB], F32, tag="mm", name="mm")
                       for _ in range(inner)]
                for t in range(pairs):
                    for i in range(inner):
                        m, tb = (i, o) if tb_outer else (o, i)
                        nc.tensor.matmul(
                            pss[i], w8[t][:, :, m * P:(m + 1) * P],
                            src8[t][:, :, tb * NB:(tb + 1) * NB],
                            start=(t == 0), stop=(t == pairs - 1),
                            perf_mode=DR)
                for i in range(inner):
                    m, tb = (i, o) if tb_outer else (o, i)
                    dst = dst8[m // 2][:, m % 2, tb * NB:(tb + 1) * NB]
                    nc.scalar.activation(
                        dst, pss[i], func,
                        bias=0.0 if bias_sb is None else bias_sb[:, m:m + 1],
                        scale=1.0)

        # ------ persistent attention operands + y prefetch ------
        with tc.tile_pool(name="pz", bufs=1) as pz, \
             tc.tile_pool(name="py", bufs=1) as py, \
             tc.tile_pool(name="pyv", bufs=1) as pyv, \
             tc.tile_pool(name="pwv", bufs=1) as pwv:
            z8 = alloc_pairs(pz, PD, S, "z8")
            y8 = alloc_pairs(py, PD, S, "y8")
            yv8t = alloc_pairs(pyv, NKV2, D, "yv8t")
            wv8 = alloc_pairs(pwv, PD, D, "wv8")

            # ---------------- Stage A: x-MLP -> z8 (in SBUF) ----------------
            with tc.tile_pool(name="wx", bufs=1) as wx, \
                 tc.tile_pool(name="px", bufs=1) as px, \
                 tc.tile_pool(name="phA", bufs=1) as phA, \
                 tc.tile_pool(name="phB", bufs=1) as phB:
                x8 = alloc_pairs(px, PD, S, "x8")
                w18 = alloc_pairs(wx, PD, D, "w18")
                # L1 runs token-block-outer: block tb only needs x8[*][tb],
                # so land full W1 + the first x8 block first, then stream
                # the remaining x8 blocks ahead of their use.
                x8r = x8d.rearrange("k (r s) -> k r s", r=2)
                for t in range(PD):
                    sl = slice(t * P, (t + 1) * P)
                    nc.sync.dma_start(
                        out=w18[t], in_=W1d[sl, :].rearrange(
                            "p (r s) -> p r s", r=2))
                    nc.sync.dma_start(out=x8[t][:, :, 0:NB],
                                      in_=x8r[sl, :, 0:NB])
                for tb in range(1, NBLK):
                    for t in range(PD):
                        sl = slice(t * P, (t + 1) * P)
                        nc.sync.dma_start(
                            out=x8[t][:, :, tb * NB:(tb + 1) * NB],
                            in_=x8r[sl, :, tb * NB:(tb + 1) * NB])
                w28 = alloc_pairs(wx, PD, DB, "w28")
                load_pairs(w28, W2d, DB)
                w38 = alloc_pairs(wx, PB, D, "w38")
                load_pairs(w38, W3d, D)
                w48 = alloc_pairs(wx, PD, D, "w48")
                load_pairs(w48, W4d, D)
                a8 = alloc_pairs(wx, PD, D, "a8")
                load_pairs(a8, A8d, D)
                # y-side prefetch (queued behind stage A's needs)
                load_pairs(y8, y8d, S)
                load_pairs(wv8, Wvd, D)

                h18 = alloc_pairs(phA, PD, S, "ha")
                h28 = alloc_pairs(phB, PB, S, "hb")
                h38 = alloc_pairs(phA, PD, S, "ha")   # reuse phA slots
                h48 = alloc_pairs(phB, PD, S, "hb")   # grow phB to 4 pair slots
                fm_layer8(psA, x8, w18, PD, KD, b1_sb, AF.Relu, h18,
                          tb_outer=True)
                fm_layer8(psA, h18, w28, PD, KB, b2_sb, AF.Relu, h28)
                fm_layer8(psA, h28, w38, PB, KD, b3_sb, AF.Relu, h38)
                fm_layer8(psA, h38, w48, PD, KD, b4_sb, AF.Relu, h48)
                fm_layer8(psA, h48, a8, PD, KD, None, AF.Identity, z8)

                # yv = y@Wv (fp8, requant to kv-pairs) -- same psum rotation
                # as the MLP (same pool+tag) so there is no bank-handoff
                # bubble at the z->yv phase boundary.
                for tkv in range(NT):
                    for db in range(2):
                        pv = psA.tile([P, NB], F32, tag="mm", name="mm")
                        for t in range(PD):
                            nc.tensor.matmul(
                                pv, y8[t][:, :, tkv * P:(tkv + 1) * P],
                                wv8[t][:, :, db * NB:(db + 1) * NB],
                                start=(t == 0), stop=(t == PD - 1),
                                perf_mode=DR)
                        nc.vector.tensor_copy(
                            out=yv8t[tkv // 2][:, tkv % 2,
                                               db * NB:(db + 1) * NB],
                            in_=pv)

            # ---------------- Stage C: attention (single pass) ----------------
            with tc.tile_pool(name="pE", bufs=1) as pE, \
                 tc.tile_pool(name="pT", bufs=10) as pT:
                et1 = alloc_pairs(pE, NKV2, S, "e")
                # scores^T -> E-1 in fp8, kv pair-packed, per q block
                for qb in range(NBLK):
                    for tk in range(NT):
                        ps = psA.tile([P, NB], F32, tag="mm", name="mm")
                        for t in range(PD):
                            nc.tensor.matmul(
                                ps, y8[t][:, :, tk * P:(tk + 1) * P],
                                z8[t][:, :, qb * NB:(qb + 1) * NB],
                                start=(t == 0), stop=(t == PD - 1),
                                perf_mode=DR)
                        etmp = pT.tile([P, NB], BF, tag="et", name="et")
                        nc.scalar.activation(etmp, ps, AF.Exp,
                                             bias=wb_sb[:, tk:tk + 1],
                                             scale=SCALE)
                        nc.vector.tensor_scalar_add(
                            et1[tk // 2][:, tk % 2, qb * NB:(qb + 1) * NB],
                            etmp, -1.0)
                for qb in range(NBLK):
                    # rowsum(E) = 2048 + sum(E-1), replicated on all
                    # partitions via the all-ones stationary
                    prs = psA.tile([P, NB], F32, tag="mm", name="mm")
                    for t2 in range(NKV2):
                        nc.tensor.matmul(
                            prs, ones8,
                            et1[t2][:, :, qb * NB:(qb + 1) * NB],
                            start=(t2 == 0), stop=(t2 == NKV2 - 1),
                            perf_mode=DR)
                    # rowsum = S + s with |s| < ~4, so one Newton step from
                    # 1/S is exact to ~2e-6: rinv = 1/S - s/S^2. Replaces a
                    # 3.3us DVE reciprocal with one cheap tensor_scalar.
                    rrep = rpool.tile([P, NB], F32, tag=f"rp{qb % 2}",
                                      name=f"rp{qb % 2}")
                    nc.vector.tensor_scalar(rrep, prs, -1.0 / (S * S),
                                            1.0 / S, op0=MULT, op1=ADD)
                    # out^T = ((E-1)@yv + c) * rinv + bv
                    for dt_ in range(KD):
                        po = psA.tile([P, NB], F32, tag="mm", name="mm")
                        for t2 in range(NKV2):
                            nc.tensor.matmul(
                                po, yv8t[t2][:, :, dt_ * P:(dt_ + 1) * P],
                                et1[t2][:, :, qb * NB:(qb + 1) * NB],
                                start=(t2 == 0), stop=(t2 == NKV2 - 1),
                                perf_mode=DR)
                        tmp = pT.tile([P, NB], F32, tag="tm", name="tm")
                        nc.vector.scalar_tensor_tensor(
                            tmp, po, cb_sb[:, dt_:dt_ + 1], rrep,
                            op0=ADD, op1=MULT)
                        ot = outp.tile([P, NB], F32, tag="ot", name="ot")
                        nc.scalar.activation(ot, tmp, AF.Identity,
                                             bias=bv_sb[:, dt_:dt_ + 1],
                                             scale=1.0)
                        nc.sync.dma_start(
                            out=outT[dt_ * P:(dt_ + 1) * P,
                                     qb * NB:(qb + 1) * NB],
                            in_=ot)

    nc.compile()
    _NC = nc
    return nc


def _pack8(w):
    """[K, N] -> DoubleRow pair-packed fp8 [K/2, 2N]:
    out[t*128+p, r*N+m] = w[(2t+r)*128+p, m]."""
    K, N = w.shape
    return np.ascontiguousarray(
        w.astype(FP8).reshape(K // 256, 2, 128, N)
        .transpose(0, 2, 1, 3).reshape(K // 2, 2 * N))


def make_in_maps(inputs):
    """Host-side prep: per-core batch shard, fp8 casts + pair packing,
    feature-major transposes of x/y, weight folding A = Wq@Wk^T, and the
    exact O(S*D) correction vectors w = SCALE*(y@(Wk@bq)), c = colsum(y)@Wv.
    """
    x = np.asarray(inputs["x"])
    y = np.asarray(inputs["y"])
    Wq = np.asarray(inputs["Wq"]).astype(np.float64)
    Wk = np.asarray(inputs["Wk"]).astype(np.float64)
    Wv = np.asarray(inputs["Wv"]).astype(np.float64)
    bq = np.asarray(inputs["bq"]).astype(np.float64)
    A = (Wq @ Wk.T).astype(np.float32)
    wk_bq = Wk @ bq
    shared = {}
    for k in ("W1", "W2", "W3", "W4"):
        shared[k] = _pack8(np.asarray(inputs[k]).astype(np.float32))
    shared["A8"] = _pack8(A)
    shared["Wv8"] = _pack8(np.asarray(inputs["Wv"]).astype(np.float32))
    for k, nt in (("b1", KD), ("b2", KB), ("b3", KD), ("b4", KD)):
        shared[k] = np.ascontiguousarray(
            np.asarray(inputs[k]).astype(np.float32).reshape(nt, P).T)
    shared["bvb"] = np.ascontiguousarray(
        np.asarray(inputs["bv"]).astype(np.float32).reshape(KD, P).T)
    in_maps = []
    for b in range(x.shape[0]):
        m = dict(shared)
        m["x8"] = _pack8(np.ascontiguousarray(x[b].T))
        m["y8"] = _pack8(np.ascontiguousarray(y[b].T))
        yb = y[b].astype(np.float64)
        m["wb"] = np.ascontiguousarray(
            (SCALE * (yb @ wk_bq)).astype(np.float32).reshape(NT, P).T)
        m["cb"] = np.ascontiguousarray(
            (yb.sum(0) @ Wv).astype(np.float32).reshape(KD, P).T)
        in_maps.append(m)
    return in_maps


def kernel(**inputs):
    from concourse.bass_utils import run_bass_kernel_spmd

    nc = build_nc()
    in_maps = make_in_maps(inputs)
    res = run_bass_kernel_spmd(nc, in_maps, list(range(len(in_maps))))
    return np.stack([np.asarray(r["outT"], dtype=np.float32).T
                     for r in res.results])
